# revision 3
# baseline (speedup 1.0000x reference)
"""Trainium2 Bass kernel v2 for nn_MultiHeadDilatedState (B=4, S=4096, H=768).

Sharding: 8 cores = (batch-pair bp) x (head-group g of 3 heads, one memory
head each): g0={6,2,10} g1={7,4,5} g2={8,3,11} g3={9,0,1}.  Each core
processes its 3 heads for BOTH batches of its pair, tiles packed
[128 = b0 64ch | b1 64ch, S].  Conv tap lags are per-core DATA via dynamic
register offsets (no cross-group union work); padded zero regions left of
each conv src tile absorb the causal prefix.  Taps run on PE as diagonal
matmul PSUM accumulations.  Exchange is two half-sequence AllToAlls
(256-token windows per dest core) overlapping the mixing matmuls.

Self-contained: hardcodes all shapes; builds + compiles once per process.
"""
import numpy as np

DILATIONS = [(1, 2, 4), (1, 1, 1), (4, 8, 16), (8, 16, 32), (32, 64, 128),
             (64, 128, 256), (256, 512, 1024), (1, 100, 200), (1, 500, 1000),
             (1, 1024, 2048), (3, 9, 27), (5, 25, 125)]
MEM_HEADS = (6, 7, 8, 9)
HIDDEN = 768
B, S = 4, 4096
N_CORES = 8
NB = HIDDEN // 128   # 6

SLOT_HEADS = [[6, 7, 8, 9], [2, 4, 3, 0], [10, 5, 11, 1]]  # [slot][group]
PADS = [3072, 384, 768]
PERM_HEADS = [SLOT_HEADS[s][g] for g in range(4) for s in range(3)]

_CACHE = {}
CONV_LAYERS = 3
SEPARATE_CV = False


def _build_bass(reps=1, ph=6):
    import concourse.bacc as bacc
    import concourse.mybir as mybir
    import concourse.tile as tile
    from concourse.ap import AP

    f32 = mybir.dt.float32
    f16 = mybir.dt.float16
    i32 = mybir.dt.int32
    AF = mybir.ActivationFunctionType
    OP = mybir.AluOpType

    nc = bacc.Bacc("TRN2", target_bir_lowering=False, debug=False,
                   num_devices=N_CORES)

    def din(name, shape, dt=f32):
        return nc.dram_tensor(name, shape, dt, kind="ExternalInput").ap()

    xT_d = din("xT", [HIDDEN, 2 * S], f16)          # cols: [b0 S | b1 S]
    wgT_d = din("wgT", [HIDDEN, 392], f16)          # gate+router out cols
    rb_d = din("rb", [8, 1])                        # router bias rows 0:3
    cdg_d = din("conv_diag", [128, 36 * 128], f16)  # 36 diag [128,128] mats
    cbi_d = din("conv_bias", [128, 9])
    lago_d = din("lag_off", [32, 8], i32)           # [tap, chunk] col offs
    qbd_d = din("mem_qbd", [128, 128], f16)
    kvg_d = din("mem_kvg", [128, 386], f16)
    gbb_d = din("mem_gb_bc", [128, 2])
    wot_d = din("mem_WoT", [128, 256], f16)
    ones_d = din("ones64", [128, 64])
    eind_d = din("E_ind", [64, 384], f16)
    onesr_d = din("ones_row", [1, 128], f16)
    idm_d = din("ident128", [128, 128], f16)
    mxbr_d = din("mixb_row", [1, HIDDEN], f16)
    mgT_d = din("mixgT", [HIDDEN, HIDDEN], f16)
    mgb_d = din("mixgb", [HIDDEN, 1])
    mxT_d = din("mixT", [HIDDEN, HIDDEN], f16)
    y_d = nc.dram_tensor("y", [4 * 512, HIDDEN], f16,
                         kind="ExternalOutput").ap()

    with tile.TileContext(nc) as tc:
        with (
            tc.tile_pool(name="const", bufs=1) as constp,
            tc.tile_pool(name="main", bufs=1) as mainp,
            tc.tile_pool(name="xt", bufs=2) as xtp,
            tc.tile_pool(name="tmp", bufs=2) as tmpp,
            tc.tile_pool(name="ps", bufs=2, space="PSUM") as psp,
            tc.tile_pool(name="dram", bufs=1, space="DRAM") as dramp,
        ):
            # ---------------- resident weights / constants ----------------
            wg_sb = [constp.tile([128, 392], f16, name=f"wg{i}")
                     for i in range(NB)]
            for i in range(NB):
                nc.sync.dma_start(wg_sb[i][:], wgT_d[128 * i:128 * (i + 1), :])
            rb_sb = constp.tile([8, 1], f32, name="rb")
            nc.sync.dma_start(rb_sb[:], rb_d[:])
            cdg_sb = constp.tile([128, 36 * 128], f16, name="cdg")
            cbi_sb = constp.tile([128, 9], f32, name="cbi")
            nc.sync.dma_start(cbi_sb[:], cbi_d[:])
            lago_sb = constp.tile([32, 8], i32, name="lago")
            nc.sync.dma_start(lago_sb[:], lago_d[:])
            qbd_sb = constp.tile([128, 128], f16, name="qbd")
            nc.sync.dma_start(qbd_sb[:], qbd_d[:])
            kvg_sb = constp.tile([128, 386], f16, name="kvgw")
            nc.sync.dma_start(kvg_sb[:], kvg_d[:])
            gbb_sb = constp.tile([128, 2], f32, name="gbb")
            nc.sync.dma_start(gbb_sb[:], gbb_d[:])
            wot_sb = constp.tile([128, 256], f16, name="wot")
            nc.sync.dma_start(wot_sb[:], wot_d[:])
            ones_sb = constp.tile([128, 64], f32, name="ones")
            nc.sync.dma_start(ones_sb[:], ones_d[:])
            eind_sb = constp.tile([64, 384], f16, name="eind")
            nc.sync.dma_start(eind_sb[:], eind_d[:])
            onesr_sb = constp.tile([1, 128], f16, name="onesr")
            nc.sync.dma_start(onesr_sb[:], onesr_d[:])
            idm_sb = constp.tile([128, 128], f16, name="idm")
            nc.sync.dma_start(idm_sb[:], idm_d[:])
            mxbr_sb = constp.tile([1, HIDDEN], f16, name="mxbr")
            nc.sync.dma_start(mxbr_sb[:], mxbr_d[:])
            mgT_sb = [constp.tile([128, HIDDEN], f16, name=f"mg{i}")
                      for i in range(NB)]
            mxT_sb = [constp.tile([128, HIDDEN], f16, name=f"mx{i}")
                      for i in range(NB)]
            mgb_sb = constp.tile([128, NB], f32, name="mgb")
            for i in range(NB):
                nc.sync.dma_start(mgb_sb[:, i:i + 1],
                                  mgb_d[128 * i:128 * (i + 1), :])

            # conv chain tiles: xg -> Av -> Bv -> (Av data region); pads
            # zeroed once, never rewritten.
            xg = [mainp.tile([128, PADS[s] + S], f16, name=f"xg{s}",
                             tag=f"xg{s}") for s in range(3)]
            Av = [mainp.tile([128, PADS[s] + S], f16, name=f"av{s}",
                             tag=f"av{s}") for s in range(3)]
            Bv = [mainp.tile([128, PADS[s] + S], f16, name=f"bv{s}",
                             tag=f"bv{s}") for s in range(3)]
            if SEPARATE_CV:
                Cv = [mainp.tile([128, S], f16, name=f"cv{s}", tag=f"cv{s}")
                      for s in range(3)]
                Cc = [Cv[s][:] for s in range(3)]
            else:
                Cc = [Av[s][:, PADS[s]:PADS[s] + S] for s in range(3)]
            for s in range(3):
                nc.vector.memset(xg[s][:, 0:PADS[s]], 0.0)
                nc.vector.memset(Av[s][:, 0:PADS[s]], 0.0)
                nc.vector.memset(Bv[s][:, 0:PADS[s]], 0.0)
            hw_sb = mainp.tile([64, S], f16, name="hww", tag="hww")
            mem_o = mainp.tile([128, S], f16, name="memo", tag="memo")
            nc.vector.memset(hw_sb[:], 0.0)

            for _rep in range(reps):

              # ======== Phase 1: gate matmul + SwiGLU + router ========
              # psum block layout (cols of wgT): mA=xg[s0|s1] 0:128,
              # mB=[xg s2|gate s0] 128:256, mC=[gate s1|s2] 256:384,
              # m3=router 384:392.
              for ck in range(8):
                  for beta in range(2):
                      cs = slice(512 * ck, 512 * (ck + 1))
                      xt = [xtp.tile([128, 512], f16, name=f"xt{i}",
                                     tag=f"xt{i}") for i in range(NB)]
                      for i in range(NB):
                          nc.sync.dma_start(
                              xt[i][:],
                              xT_d[128 * i:128 * (i + 1),
                                   S * beta + 512 * ck:
                                   S * beta + 512 * (ck + 1)])
                      ps_mB = psp.tile([128, 512], f32, name="psmB", tag="A")
                      for db in range(NB):
                          nc.tensor.matmul(ps_mB[:], wg_sb[db][:, 128:256],
                                           xt[db][:], start=(db == 0),
                                           stop=(db == NB - 1))
                      ps_mC = psp.tile([128, 512], f32, name="psmC", tag="A")
                      for db in range(NB):
                          nc.tensor.matmul(ps_mC[:], wg_sb[db][:, 256:384],
                                           xt[db][:], start=(db == 0),
                                           stop=(db == NB - 1))
                      sg = [tmpp.tile([64, 512], f16, name=f"sg{s}",
                                      tag=f"sg{s}") for s in range(3)]
                      nc.scalar.activation(sg[0][:], ps_mB[64:128, :],
                                           AF.Sigmoid)
                      nc.scalar.activation(sg[1][:], ps_mC[0:64, :],
                                           AF.Sigmoid)
                      nc.scalar.activation(sg[2][:], ps_mC[64:128, :],
                                           AF.Sigmoid)
                      x2p = tmpp.tile([64, 512], f16, name="x2p", tag="x2p")
                      nc.scalar.copy(x2p[:], ps_mB[0:64, :])
                      ps_mA = psp.tile([128, 512], f32, name="psmA", tag="A")
                      for db in range(NB):
                          nc.tensor.matmul(ps_mA[:], wg_sb[db][:, 0:128],
                                           xt[db][:], start=(db == 0),
                                           stop=(db == NB - 1))
                      ps_m3 = psp.tile([8, 512], f32, name="psm3", tag="R",
                                       bufs=1)
                      for db in range(NB):
                          nc.tensor.matmul(ps_m3[:], wg_sb[db][:, 384:392],
                                           xt[db][:], start=(db == 0),
                                           stop=(db == NB - 1))
                      rows = slice(64 * beta, 64 * (beta + 1))
                      nc.vector.tensor_tensor(
                          xg[0][rows, PADS[0] + 512 * ck:
                                PADS[0] + 512 * (ck + 1)],
                          ps_mA[0:64, :], sg[0][:], OP.mult)
                      nc.vector.tensor_tensor(
                          xg[1][rows, PADS[1] + 512 * ck:
                                PADS[1] + 512 * (ck + 1)],
                          ps_mA[64:128, :], sg[1][:], OP.mult)
                      nc.vector.tensor_tensor(
                          xg[2][rows, PADS[2] + 512 * ck:
                                PADS[2] + 512 * (ck + 1)],
                          x2p[:], sg[2][:], OP.mult)
                      nc.scalar.activation(hw_sb[32 * beta:32 * beta + 3, cs],
                                           ps_m3[0:3, :], AF.Sigmoid,
                                           bias=rb_sb[0:3, 0:1], scale=1.0)

              if _rep == 0:
                  nc.sync.dma_start(cdg_sb[:], cdg_d[:])
                  for i in range(NB):
                      nc.sync.dma_start(mgT_sb[i][:],
                                        mgT_d[128 * i:128 * (i + 1), :])
                      nc.sync.dma_start(mxT_sb[i][:],
                                        mxT_d[128 * i:128 * (i + 1), :])

              # ======== Phase 2 + 3a: memory scan with interleaved conv L0
              _tapoff = {}

              def tap_off(s, lay, k):
                  key = (s, lay, k)
                  if key not in _tapoff:
                      idx = s * 9 + lay * 3 + k
                      _tapoff[key] = nc.tensor.value_load(
                          lago_sb[idx:idx + 1, 0:1])
                  return _tapoff[key]

              def conv_group(s, lay, ck, srcs, dsts, fuse_weight=False):
                  src, dst = srcs[s], dsts[s]
                  dpad = 0 if lay == 2 else PADS[s]
                  dbase = (s * 3 + lay) * 4
                  ps_c = psp.tile([128, 512], f32, name="psc", tag="F")
                  nc.tensor.matmul(
                      ps_c[:], cdg_sb[:, 128 * dbase:128 * (dbase + 1)],
                      src[:, PADS[s] + 512 * ck:PADS[s] + 512 * (ck + 1)],
                      start=True, stop=False)
                  for k in range(3):
                      sap = src[:, 0:512]
                      sdyn = AP(tensor=sap.tensor,
                                offset=sap.offset + tap_off(s, lay, k)
                                + 512 * ck,
                                ap=sap.ap)
                      nc.tensor.matmul(
                          ps_c[:], cdg_sb[:, 128 * (dbase + 1 + k):
                                          128 * (dbase + 2 + k)],
                          sdyn, start=False, stop=(k == 2 and not
                                                   (fuse_weight and s == 0)))
                  if not fuse_weight:
                      nc.scalar.activation(
                          dst[:, dpad + 512 * ck:dpad + 512 * (ck + 1)],
                          ps_c[:], AF.Identity,
                          bias=cbi_sb[:, s * 3 + lay:s * 3 + lay + 1],
                          scale=1.0)
                      return
                  cs = slice(512 * ck, 512 * (ck + 1))
                  if s == 0:
                      # += mem_o via identity matmul into the open group
                      nc.tensor.matmul(ps_c[:], idm_sb[:], mem_o[:, cs],
                                       start=False, stop=True)
                  ps_h = psp.tile([128, 512], f32, name="psh", tag="F")
                  nc.tensor.matmul(ps_h[:],
                                   eind_sb[:, 128 * s:128 * (s + 1)],
                                   hw_sb[:, cs], start=True, stop=True)
                  tmpb = tmpp.tile([128, 512], f32, name="cb", tag="cb")
                  nc.scalar.activation(
                      tmpb[:], ps_c[:], AF.Identity,
                      bias=cbi_sb[:, s * 3 + lay:s * 3 + lay + 1], scale=1.0)
                  nc.vector.tensor_tensor(dst[:, cs], tmpb[:], ps_h[:],
                                          OP.mult)

              l0_groups = [(s, 0, ck) for ck in range(8) for s in range(3)]
              l0_emitted = 0

              if ph >= 2:
                x_mem = xg[0]
                P0 = PADS[0]
                rd_ck = [mainp.tile([128, 512], f16, name=f"rdck{b}",
                                    tag=f"rdck{b}") for b in range(2)]
                M_a = mainp.tile([64, 128], f32, name="Ma", tag="Ma")
                M_b = mainp.tile([64, 128], f32, name="Mb", tag="Mb")
                nc.vector.memset(M_a[:], 0.0)
                nc.vector.memset(M_b[:], 0.0)
                for blk in range(S // 128):
                    bs = slice(P0 + 128 * blk, P0 + 128 * (blk + 1))
                    ps_q = psp.tile([64, 256], f32, name="psq", tag="Q",
                                    bufs=1)
                    nc.tensor.matmul(ps_q[:, 0:128], qbd_sb[:, 0:64],
                                     x_mem[:, bs], start=True, stop=True)
                    nc.tensor.matmul(ps_q[:, 128:256], qbd_sb[:, 64:128],
                                     x_mem[:, bs], start=True, stop=True)
                    q_ab = tmpp.tile([64, 256], f32, name="qab", tag="qab")
                    nc.scalar.copy(q_ab[:], ps_q[:])
                    q_a = q_ab[:, 0:128]
                    q_b = q_ab[:, 128:256]
                    ps_rd = psp.tile([128, 256], f32, name="psrd", tag="RD",
                                     bufs=1)
                    for half in range(2):
                        c64 = slice(P0 + 128 * blk + 64 * half,
                                    P0 + 128 * blk + 64 * (half + 1))
                        ps_kvg = psp.tile([64, 386], f32, name="pskvg",
                                          tag="A")
                        nc.tensor.matmul(ps_kvg[:], x_mem[:, c64], kvg_sb[:],
                                         start=True, stop=True)
                        g_sb = tmpp.tile([64, 2], f32, name="gsb", tag="gsb")
                        nc.scalar.activation(g_sb[:, 0:2],
                                             ps_kvg[:, 384:386],
                                             AF.Sigmoid,
                                             bias=gbb_sb[0:64, 0:1],
                                             scale=1.0)
                        kg_sb = tmpp.tile([64, 128], f16, name="kgsb",
                                          tag="kgsb")
                        for b2 in range(2):
                            nc.vector.tensor_scalar(
                                kg_sb[:, 64 * b2:64 * (b2 + 1)],
                                ps_kvg[:, 64 * b2:64 * (b2 + 1)],
                                g_sb[:, b2:b2 + 1], None, OP.mult)
                        v_sb = tmpp.tile([64, 256], f16, name="vsb",
                                         tag="vsb")
                        nc.vector.tensor_copy(v_sb[:], ps_kvg[:, 128:384])
                        nc.tensor.matmul(
                            ps_rd[:, 64 * half:64 * (half + 1)],
                            M_a[:], q_a[:, 64 * half:64 * (half + 1)],
                            start=True, stop=True)
                        nc.tensor.matmul(
                            ps_rd[:, 128 + 64 * half:128 + 64 * (half + 1)],
                            M_b[:], q_b[:, 64 * half:64 * (half + 1)],
                            start=True, stop=True)
                        ps_g = psp.tile([64, 2], f32, name="psg", tag="R",
                                        bufs=1)
                        nc.tensor.matmul(ps_g[:], ones_sb[0:64, :], g_sb[:],
                                         start=True, stop=True)
                        decay = tmpp.tile([64, 2], f32, name="decay",
                                          tag="decay")
                        nc.scalar.activation(decay[:], ps_g[:], AF.Identity,
                                             bias=1.0, scale=-1.0)
                        ps_w = psp.tile([64, 256], f32, name="psw", tag="E",
                                        bufs=1)
                        nc.tensor.matmul(ps_w[:, 0:128], kg_sb[:, 0:64],
                                         v_sb[:, 0:128], start=True,
                                         stop=True)
                        nc.tensor.matmul(ps_w[:, 128:256], kg_sb[:, 64:128],
                                         v_sb[:, 128:256], start=True,
                                         stop=True)
                        nc.vector.scalar_tensor_tensor(
                            M_a[:], M_a[:], decay[:, 0:1], ps_w[:, 0:128],
                            OP.mult, OP.add)
                        nc.vector.scalar_tensor_tensor(
                            M_b[:], M_b[:], decay[:, 1:2], ps_w[:, 128:256],
                            OP.mult, OP.add)
                    cc4 = 128 * blk % 512
                    for b2 in range(2):
                        nc.vector.tensor_copy(rd_ck[b2][:, cc4:cc4 + 128],
                                              ps_rd[:, 128 * b2:128 * (b2 + 1)])
                    if blk % 4 == 3:
                        ck4 = blk // 4
                        cs4 = slice(512 * ck4, 512 * (ck4 + 1))
                        ps_o = psp.tile([128, 512], f32, name="pso", tag="E",
                                        bufs=1)
                        nc.tensor.matmul(ps_o[:], wot_sb[:, 0:128],
                                         rd_ck[0][:], start=True, stop=False)
                        nc.tensor.matmul(ps_o[:], wot_sb[:, 128:256],
                                         rd_ck[1][:], start=False, stop=True)
                        nc.scalar.copy(mem_o[:, cs4], ps_o[:])
                    if ph >= 3:
                        while l0_emitted < (blk + 1) * 24 // 32:
                            s_, l_, c_ = l0_groups[l0_emitted]
                            conv_group(s_, l_, c_, xg, Av)
                            l0_emitted += 1

              # ======== Phase 3b: conv L1; then per-half L2 + weighting
              # + exchange (pipelined so each AllToAll overlaps compute)
              if ph >= 3:
                while l0_emitted < 24:
                    s_, l_, c_ = l0_groups[l0_emitted]
                    conv_group(s_, l_, c_, xg, Av)
                    l0_emitted += 1
                if CONV_LAYERS >= 2:
                    for ck in range(8):
                        for s in range(3):
                            conv_group(s, 1, ck, Av, Bv)

              bounce_out_h = [None, None]
              for H in range(2):
                  if ph >= 3 and CONV_LAYERS >= 3:
                      for ck in range(4 * H, 4 * H + 4):
                          for s in range(3):
                              conv_group(s, 2, ck, Bv, Cc)
                  if ph >= 4:
                      for ck in range(4 * H, 4 * H + 4):
                          cs = slice(512 * ck, 512 * (ck + 1))
                          nc.vector.tensor_tensor(Cc[0][:, cs], Cc[0][:, cs],
                                                  mem_o[:, cs], OP.add)
                      for s in range(3):
                          for ck in range(4 * H, 4 * H + 4):
                              cs = slice(512 * ck, 512 * (ck + 1))
                              ps_h = psp.tile([128, 512], f32, name="psh",
                                              tag="F")
                              nc.tensor.matmul(
                                  ps_h[:], eind_sb[:, 128 * s:128 * (s + 1)],
                                  hw_sb[:, cs], start=True, stop=True)
                              nc.vector.tensor_tensor(Cc[s][:, cs],
                                                      Cc[s][:, cs],
                                                      ps_h[:], OP.mult)
                  if ph >= 5:
                      bounce_in = dramp.tile([N_CORES * 384, 256], f16,
                                             name=f"bin{H}")
                      bounce_out = dramp.tile([N_CORES * 384, 256], f16,
                                              name=f"bout{H}")
                      bounce_out_h[H] = bounce_out
                      for j in range(N_CORES):
                          w0 = 2048 * H + 256 * j
                          for s in range(3):
                              nc.sync.dma_start(
                                  bounce_in[384 * j + 128 * s:
                                            384 * j + 128 * (s + 1), :],
                                  Cc[s][:, w0:w0 + 256])
                      nc.gpsimd.collective_compute(
                          "AllToAll", mybir.AluOpType.bypass,
                          replica_groups=[list(range(N_CORES))],
                          ins=[bounce_in[:].opt()],
                          outs=[bounce_out[:].opt()])

              if ph < 5:
                  for s in range(3):
                      nc.sync.dma_start(
                          y_d[128 * s:128 * (s + 1), :],
                          xg[s][:, PADS[s]:PADS[s] + HIDDEN])
                  if ph >= 2:
                      nc.sync.dma_start(y_d[384:512, :], mem_o[:, 0:HIDDEN])
                  if ph >= 3:
                      dbg = Bv if CONV_LAYERS == 2 else [Cc[s_2] for s_2 in
                                                         range(3)]
                      for s in range(3):
                          dsrc = (Bv[s][:, PADS[s]:PADS[s] + HIDDEN]
                                  if CONV_LAYERS == 2
                                  else Cc[s][:, 0:HIDDEN])
                          nc.sync.dma_start(y_d[512 + 128 * s:
                                                512 + 128 * (s + 1), :],
                                            dsrc)
                  nc.sync.dma_start(y_d[896:960, :], hw_sb[:, 0:HIDDEN])
                  continue
              # ======== Phase 6: mixing ========
              for H in range(2):
                  bounce_out = bounce_out_h[H]
                  hT = [mainp.tile([128, 1024], f16, name=f"ht{i}",
                                   tag=f"ht{i}") for i in range(NB)]
                  for b in range(B):
                      bp, beta = b // 2, b % 2
                      for g in range(4):
                          q = 4 * bp + g
                          for s in range(3):
                              fpos = 192 * g + 64 * s
                              fb, fr = fpos // 128, fpos % 128
                              nc.sync.dma_start(
                                  hT[fb][fr:fr + 64, 256 * b:256 * (b + 1)],
                                  bounce_out[384 * q + 128 * s + 64 * beta:
                                             384 * q + 128 * s + 64 * beta
                                             + 64, :])
                  if ph < 6:
                      nc.sync.dma_start(y_d[1024 * H:1024 * H + 128, 0:768],
                                        hT[0][:, 0:768])
                      continue
                  for tck in range(2):
                      cs = slice(512 * tck, 512 * (tck + 1))
                      sigs = []
                      for fb in range(NB):
                          ps_pre = psp.tile([128, 512], f32, name="pre",
                                            tag="A")
                          for db in range(NB):
                              nc.tensor.matmul(
                                  ps_pre[:],
                                  mgT_sb[db][:, 128 * fb:128 * (fb + 1)],
                                  hT[db][:, cs], start=(db == 0),
                                  stop=(db == NB - 1))
                          sgm = tmpp.tile([128, 512], f16, name=f"msig{fb}",
                                          tag=f"msig{fb}", bufs=1)
                          nc.scalar.activation(sgm[:], ps_pre[:], AF.Sigmoid,
                                               bias=mgb_sb[:, fb:fb + 1],
                                               scale=1.0)
                          sigs.append(sgm)
                      for fb in range(NB):
                          nc.vector.tensor_tensor(hT[fb][:, cs],
                                                  hT[fb][:, cs],
                                                  sigs[fb][:], OP.mult)
                      for tb in range(4):
                          tr = slice(512 * tck + 128 * tb,
                                     512 * tck + 128 * (tb + 1))
                          yrow = 1024 * H + 512 * tck + 128 * tb
                          for half in range(2):
                              ps_y = psp.tile([128, 384], f32, name="psy",
                                              tag="F")
                              for fb in range(NB):
                                  nc.tensor.matmul(
                                      ps_y[:], hT[fb][:, tr],
                                      mxT_sb[fb][:, 384 * half:
                                                  384 * (half + 1)],
                                      start=(fb == 0), stop=False)
                              nc.tensor.matmul(
                                  ps_y[:], onesr_sb[:],
                                  mxbr_sb[:, 384 * half:384 * (half + 1)],
                                  start=False, stop=True)
                              # BIAS_MM_MARK
                              y_sb = tmpp.tile([128, 384], f16, name="ysb",
                                               tag=f"ysb{half}")
                              nc.scalar.copy(y_sb[:], ps_y[:])
                              nc.sync.dma_start(
                                  y_d[yrow:yrow + 128,
                                      384 * half:384 * (half + 1)],
                                  y_sb[:])

    nc.compile()
    return nc


def _prep_core_inputs(core, inp):
    bp, g = core // 4, core % 4
    b0, b1 = 2 * bp, 2 * bp + 1
    heads = [SLOT_HEADS[s][g] for s in range(3)]
    f32, f16 = np.float32, np.float16

    x = np.asarray(inp["x"], f32)                     # [B, S, H]
    xT = np.concatenate([x[b0].T, x[b1].T], axis=1)   # [H, 2S]

    gate_w = np.asarray(inp["gate_w"], f32)
    router_w = np.asarray(inp["router_w"], f32)
    router_b = np.asarray(inp["router_b"], f32)
    wg = np.zeros((HIDDEN, 392), f32)
    for s, h in enumerate(heads):
        wg[:, 64 * s:64 * (s + 1)] = gate_w[64 * h:64 * (h + 1)].T
        wg[:, 192 + 64 * s:192 + 64 * (s + 1)] = \
            gate_w[768 + 64 * h:768 + 64 * (h + 1)].T
        wg[:, 384 + s] = router_w[h]
    rb = np.zeros((8, 1), f32)
    rb[0:3, 0] = router_b[heads]

    conv_w = np.asarray(inp["conv_w"], f32)
    conv_b = np.asarray(inp["conv_b"], f32)
    cdg = np.zeros((128, 36 * 128), f32)
    cbi = np.zeros((128, 9), f32)
    lago = np.zeros((32, 8), np.int32)
    for s, h in enumerate(heads):
        for lay in range(3):
            dbase = (s * 3 + lay) * 4
            d = DILATIONS[h][lay]
            w_s1 = 1.0 + conv_w[h, lay, :, 3]
            for r in range(128):
                cdg[r, 128 * dbase + r] = w_s1[r % 64]
            for k in range(1, 4):
                lag = k * d
                wk = conv_w[h, lay, :, 3 - k] if lag < S else \
                    np.zeros(64, f32)
                for r in range(128):
                    cdg[r, 128 * (dbase + k) + r] = wk[r % 64]
                lag_eff = lag if lag < S else 0
                tap = s * 9 + lay * 3 + (k - 1)
                for ck in range(8):
                    lago[tap, ck] = PADS[s] + 512 * ck - lag_eff
            cbi[0:64, s * 3 + lay] = conv_b[h, lay]
            cbi[64:128, s * 3 + lay] = conv_b[h, lay]

    m = g  # memory head index within MEM_HEADS
    Wq = np.asarray(inp["mem_Wq"], f32)[m]
    Wk = np.asarray(inp["mem_Wk"], f32)[m]
    Wv = np.asarray(inp["mem_Wv"], f32)[m]
    Wgw = np.asarray(inp["mem_Wg_w"], f32)[m]
    Wgb = np.asarray(inp["mem_Wg_b"], f32)[m]
    Wo = np.asarray(inp["mem_Wout"], f32)[m]

    qbd = np.zeros((128, 128), f32)
    qbd[0:64, 0:64] = Wq.T
    qbd[64:128, 64:128] = Wq.T
    kvg = np.zeros((128, 386), f32)
    kvg[0:64, 0:64] = Wk.T
    kvg[64:128, 64:128] = Wk.T
    kvg[0:64, 128:256] = Wv.T
    kvg[64:128, 256:384] = Wv.T
    kvg[0:64, 384] = Wgw[0]
    kvg[64:128, 385] = Wgw[0]
    gbb = np.zeros((128, 2), f32)
    gbb[:, 0] = Wgb[0]
    gbb[:, 1] = Wgb[0]
    wot = np.zeros((128, 256), f32)
    wot[:, 0:64] = Wo.T
    wot[:, 192:256] = Wo.T

    eind = np.zeros((64, 384), f32)
    for s in range(3):
        for p in range(128):
            eind[32 * (p // 64) + s, 128 * s + p] = 1.0

    pf = np.concatenate([np.arange(64 * h, 64 * h + 64)
                         for h in PERM_HEADS])
    mixg_w = np.asarray(inp["mixg_w"], f32)
    mix_w = np.asarray(inp["mix_w"], f32)

    return {
        "xT": xT.astype(f16),
        "wgT": wg.astype(f16), "rb": rb,
        "conv_diag": cdg.astype(f16), "conv_bias": cbi, "lag_off": lago,
        "mem_qbd": qbd.astype(f16), "mem_kvg": kvg.astype(f16),
        "mem_gb_bc": gbb, "mem_WoT": wot.astype(f16),
        "ones64": np.full((128, 64), 1.0 / 64.0, f32),
        "E_ind": eind.astype(f16),
        "ones_row": np.ones((1, 128), f16),
        "ident128": np.eye(128, dtype=f16),
        "mixb_row": np.asarray(inp["mix_b"], f32)[None, :].astype(f16),
        "mixgT": np.ascontiguousarray(
            mixg_w[np.ix_(pf, pf)].T).astype(f16),
        "mixgb": np.asarray(inp["mixg_b"], f32)[pf].reshape(HIDDEN, 1).copy(),
        "mixT": np.ascontiguousarray(mix_w[:, pf].T).astype(f16),
    }


def prep_in_maps(inputs):
    return [_prep_core_inputs(c, inputs) for c in range(N_CORES)]


def get_bass():
    if "nc" not in _CACHE:
        _CACHE["nc"] = _build_bass()
    return _CACHE["nc"]


def assemble(results):
    out = np.zeros((B, S, HIDDEN), np.float32)
    for j in range(N_CORES):
        y = np.asarray(results[j]["y"], np.float32)
        for H in range(2):
            for b in range(B):
                w0 = 2048 * H + 256 * j
                out[b, w0:w0 + 256, :] = y[1024 * H + 256 * b:
                                           1024 * H + 256 * (b + 1)]
    return out


def kernel(**inputs):
    from concourse import bass_utils
    nc = get_bass()
    in_maps = prep_in_maps(inputs)
    res = bass_utils.run_bass_kernel_spmd(nc, in_maps,
                                          core_ids=list(range(N_CORES)))
    return assemble(res.results)
